# revision 46
# baseline (speedup 1.0000x reference)
# Multi-head self-attention (B=2, T=2048, C=2048, H=16) on 8 trn2 NeuronCores.
# Sharding: core = (batch b, head-group g) with 4 heads per core.
# Inputs are pre-cast to bf16 and packed DMA-friendly on the host (the device
# would do the identical round-to-nearest cast before its bf16 matmuls).
# Per-core program (Tile framework, bf16 matmuls with fp32 PSUM accumulation):
#   warmup matmuls on a zero tile pre-warm the PE HAM clock gate while DMA
#     rings spin up
#   qk^T = W_qk^T @ x^T   (lhsT = W chunks, rhs = x^T)      -> [D, T] per head
#   v    = x @ W_v        (lhsT = x^T chunks, rhs = W_v)    -> [T, D] natural
#   RoPE on q^T/k^T via half-swap DMA + elementwise mul/add (in place)
#   merged attention+projection phase, qo-major [3,2,1,0]:
#     S^T tile = k_rope^T.T @ q_rope^T ; E^T = exp(scale*S^T) (causal)
#     out^T = v.T @ E^T ; rowsums: DVE-accumulate E tiles in fp32 SBUF,
#       one ones-matmul per (h,qo) replicates sums to all partitions
#     normalize off the PSUM path: evac unscaled, scale by 1/sums in SBUF
#     proj tiles of block qo interleave with attention of block qo-1 so the
#       PE stream stays dense and the y out-DMA is spread across the phase
#   y_partial = out_heads^T.T @ W_p rows  -> [T, C] fp16, host sums 4 partials.
import sys

import numpy as np
import ml_dtypes

for _p in ("/opt/trn_rl_repo",):
    if _p not in sys.path:
        sys.path.append(_p)

import concourse.bass as bass
import concourse.bass_isa as bass_isa
import concourse.mybir as mybir
import concourse.tile as tile
from concourse import bacc
from concourse.bass_utils import run_bass_kernel_spmd

P = 128
T = 2048
C = 2048
D = 128
NH = 4            # heads per core
KO = C // P       # 16 contraction chunks
TQ = 512          # q-tile width
NQ = T // TQ      # 4
NT = T // P       # 16 t-subtiles
SCALE = float(np.float32(1.0) / np.sqrt(np.float32(D)))

F32 = mybir.dt.float32
F16 = mybir.dt.float16
BF16 = mybir.dt.bfloat16
AF = mybir.ActivationFunctionType
BF = ml_dtypes.bfloat16

TRACE = False
_CACHED_NC = None


def _tri_mask_np():
    p = np.arange(P)[:, None]
    q = np.arange(P)[None, :]
    return (p <= q).astype(BF)


def build_nc():
    nc = bacc.Bacc("TRN2", target_bir_lowering=False, debug=False,
                   enable_asserts=False)

    # bf16 inputs, packed so every DMA moves >=4KB contiguous per partition
    xT_d = nc.dram_tensor("xT", [C, T], BF16, kind="ExternalInput")
    wqk_d = nc.dram_tensor("wqk", [8, P, KO, P], BF16, kind="ExternalInput")
    wv_d = nc.dram_tensor("wv", [P, KO, NH * D], BF16, kind="ExternalInput")
    wp_d = nc.dram_tensor("wp", [P, NH, C], BF16, kind="ExternalInput")
    cos_d = nc.dram_tensor("cosT", [D, T], BF16, kind="ExternalInput")
    sin_d = nc.dram_tensor("sinT", [D, T], BF16, kind="ExternalInput")
    y_d = nc.dram_tensor("y", [T, C], F16, kind="ExternalOutput")

    mask_d = nc.inline_tensor(_tri_mask_np(), name="trimask")

    xT = xT_d.ap().rearrange("(ko p) t -> p ko t", p=P)          # [128,16,2048]
    wqk = wqk_d.ap()
    y = y_d.ap()

    with tile.TileContext(nc) as tc:
        with (
            tc.tile_pool(name="glob", bufs=1) as glob,
            tc.tile_pool(name="rawp", bufs=1) as rawp,
        ):
            # HAM warmup: ~20 N=256 matmuls on a zeroed tile keep the PE busy
            # from ~t=2us so the clock gate is at 8/8 before real work arrives
            warm_sb = glob.tile([P, 256], BF16, tag="warm")
            nc.vector.memset(warm_sb[:], 0.0)
            ones_sb = glob.tile([P, P], BF16, tag="ones")
            nc.vector.memset(ones_sb[:], 1.0)
            v_b = [glob.tile([P, NT, P], BF16, tag=f"v_b{h}", name=f"v_b{h}")
                   for h in range(NH)]
            raw = [rawp.tile([P, T], BF16, tag=f"raw{m}", name=f"raw{m}")
                   for m in range(8)]
            mask_sb = glob.tile([P, P], BF16, tag="trimask")
            wp_b = glob.tile([P, NH, C], BF16, tag="wp_b")

            # =============== Phase B: qkv matmuls + RoPE ===============
            # warmps persists through phase B: its bank hosts the HAM-warmup
            # matmuls plus no-op filler matmuls that soak up PE idle while the
            # x chunks stream in
            with tc.tile_pool(name="warmps", bufs=1, space="PSUM") as wps, \
                 tc.tile_pool(name="loadB", bufs=1) as lB, \
                 tc.tile_pool(name="shufp", bufs=1) as shufp, \
                 tc.tile_pool(name="psB", bufs=7, space="PSUM") as psB:

                w_ps = wps.tile([P, 256], F32, tag="wps")
                for _ in range(17):
                    nc.tensor.matmul(w_ps[:], lhsT=warm_sb[:, 0:P],
                                     rhs=warm_sb[:], start=True, stop=True)

                def filler(dep=None, n=256):
                    # anchor on `dep` so the scheduler places the filler where
                    # that data lands instead of hoisting it to t=0
                    rhs = warm_sb[:, 0:n] if dep is None else dep
                    nc.tensor.matmul(w_ps[:, 0:rhs.shape[-1]],
                                     lhsT=warm_sb[:, 0:P],
                                     rhs=rhs, start=True, stop=True)

                xb = lB.tile([P, KO, T], BF16, tag="xb")
                wqk_b = [lB.tile([P, KO, P], BF16, tag=f"wqk_b{m}",
                                 name=f"wqk_b{m}") for m in range(8)]
                # x is the critical stream: only the first two m-groups'
                # weights jump the queue (groups 0-1 are all the PE can keep
                # in flight while x lands); everything else loads after x
                nc.sync.dma_start(wqk_b[0][:, 0:4, :], wqk[0, :, 0:4, :])
                nc.sync.dma_start(xb[:, 0, :], xT[:, 0, :])
                nc.sync.dma_start(wqk_b[0][:, 4:KO, :], wqk[0, :, 4:KO, :])
                nc.sync.dma_start(xb[:, 1, :], xT[:, 1, :])
                nc.sync.dma_start(wqk_b[1][:], wqk[1])
                for ko in range(2, KO):
                    nc.sync.dma_start(xb[:, ko, :], xT[:, ko, :])
                for m in range(2, 8):
                    nc.sync.dma_start(wqk_b[m][:], wqk[m])
                cos_b = lB.tile([P, T], BF16, tag="cos_b")
                sin_b = lB.tile([P, T], BF16, tag="sin_b")
                nc.sync.dma_start(cos_b[:], cos_d.ap())
                nc.sync.dma_start(sin_b[:], sin_d.ap())
                wv_b = lB.tile([P, KO, NH * D], BF16, tag="wv_b")
                nc.sync.dma_start(wv_b[:], wv_d.ap())
                nc.sync.dma_start(mask_sb[:], mask_d.ap())
                # wp is only needed by the first proj tile (~mid-kernel); queue
                # it after every qkv input so it can't delay the x/wqk stream
                nc.sync.dma_start(wp_b[:], wp_d.ap())

                # qk matmuls; k-order staggered by m so each group consumes
                # chunks roughly in DMA-arrival order
                for m in range(8):
                    pss = [psB.tile([P, TQ], F32, tag="psBig", name="psqk")
                           for _ in range(NQ)]
                    kos = [(m + i) % KO for i in range(KO)]
                    for i, ko in enumerate(kos):
                        for n in range(NQ):
                            nc.tensor.matmul(
                                pss[n][:], lhsT=wqk_b[m][:, ko, :],
                                rhs=xb[:, ko, n * TQ:(n + 1) * TQ],
                                start=(i == 0), stop=(i == KO - 1))
                        if m < 2 and i % 2 == 1:
                            filler(dep=xb[:, ko, 0:256])
                    for n in range(NQ):
                        nc.scalar.activation(
                            raw[m][:, n * TQ:(n + 1) * TQ], pss[n][:], AF.Copy)
                    # rope in place
                    r = raw[m]
                    sh = shufp.tile([P, T], BF16, tag="shuf", name="sh")
                    nc.sync.dma_start(sh[0:64, :], r[64:128, :])
                    nc.sync.dma_start(sh[64:128, :], r[0:64, :])
                    nc.vector.tensor_mul(sh[:], sh[:], sin_b[:])
                    nc.vector.tensor_mul(r[:], r[:], cos_b[:])
                    nc.vector.tensor_add(r[:], r[:], sh[:])

                # v matmuls (natural layout)
                for t in range(NT):
                    psv = psB.tile([P, TQ], F32, tag="psBig", name="psv")
                    for ko in range(KO):
                        nc.tensor.matmul(
                            psv[:], lhsT=xb[:, ko, t * P:(t + 1) * P],
                            rhs=wv_b[:, ko, :],
                            start=(ko == 0), stop=(ko == KO - 1))
                    for h in range(NH):
                        # split the last tiles' evacs across engines so the
                        # PSUM pool handoff to the attention phase is quick
                        if t >= NT - 2 and h < 2:
                            nc.vector.tensor_copy(
                                v_b[h][:, t, :], psv[:, h * P:(h + 1) * P])
                        else:
                            nc.scalar.activation(
                                v_b[h][:, t, :], psv[:, h * P:(h + 1) * P],
                                AF.Copy)
                # keep the PE busy across the pool transition into attention;
                # anchored on the last v tiles so these run at the handoff
                for h in range(NH):
                    filler(dep=v_b[h][:, NT - 1, :])
                    filler(dep=v_b[h][:, NT - 2, :])

            # =============== Merged attention + projection ===============
            with tc.tile_pool(name="attnp", bufs=1) as ap_, \
                 tc.tile_pool(name="etp", bufs=3) as etp, \
                 tc.tile_pool(name="nrm", bufs=3) as nrm, \
                 tc.tile_pool(name="psS2", bufs=2, space="PSUM") as psS2, \
                 tc.tile_pool(name="psO", bufs=2, space="PSUM") as psO, \
                 tc.tile_pool(name="psPj", bufs=2, space="PSUM") as psPj, \
                 tc.tile_pool(name="ystg", bufs=3) as ystg:

                # outT per qo block: [P(d), NH, TQ]; double-buffered so proj
                # of block qo overlaps attention of block qo-1
                outT = {}

                def attn_block(h, qo, proj_emit=None):
                    # Software-pipelined block: scores for chunk-pair p+1 are
                    # emitted before the AV matmuls of pair p, so each exp's
                    # ~1.2us latency hides under the next pair's scores; the
                    # interleaved proj tile is emitted right after the first
                    # scores pair and absorbs its exp latency at block entry.
                    qr = raw[h]
                    kr = raw[4 + h]
                    qsl = slice(qo * TQ, (qo + 1) * TQ)
                    nfull = 4 * qo
                    nq2 = nfull // 2
                    ps_o = psO.tile([P, TQ], F32, tag="psout", name="ps_o")
                    st = {"ps_r": None, "esum_q": None}

                    def scores_pair(pr, split_exp=False):
                        ps2 = psS2.tile([P, 2, TQ], F32, tag="psscore",
                                        name="ps2")
                        for s in range(2):
                            j = 2 * pr + s
                            nc.tensor.matmul(
                                ps2[:, s, :],
                                lhsT=kr[:, j * P:(j + 1) * P],
                                rhs=qr[:, qsl], start=True, stop=True)
                        et2 = etp.tile([P, 2, TQ], BF16, tag="et2",
                                       name="et2", bufs=5)
                        if split_exp:
                            # per-plane exps: the first AV only needs plane 0,
                            # so it can start ~half an exp earlier (used at
                            # block entry when no proj tile covers the wait)
                            for s in range(2):
                                nc.scalar.activation(et2[:, s, :],
                                                     ps2[:, s, :], AF.Exp,
                                                     scale=SCALE)
                        else:
                            nc.scalar.activation(et2[:], ps2[:], AF.Exp,
                                                 scale=SCALE)
                        return et2

                    def av_pair(pr, et2):
                        for s in range(2):
                            j = 2 * pr + s
                            nc.tensor.matmul(
                                ps_o[:], lhsT=v_b[h][:, j, :],
                                rhs=et2[:, s, :],
                                start=(pr == 0 and s == 0), stop=False)
                        esum = etp.tile([P, TQ], BF16, tag="esum",
                                        name="esum", bufs=6)
                        nc.vector.tensor_add(esum[:], et2[:, 0, :],
                                             et2[:, 1, :])
                        if pr % 2 == 0:
                            st["esum_q"] = esum
                        else:
                            # quad complete; batch two quads into one oct
                            # rowsum matmul when a second quad is coming
                            nc.vector.tensor_add(st["esum_q"][:],
                                                 st["esum_q"][:], esum[:])
                            if pr % 4 == 1 and pr + 2 < nq2:
                                st["esum_o"] = st["esum_q"]
                            elif pr % 4 == 3:
                                nc.vector.tensor_add(st["esum_o"][:],
                                                     st["esum_o"][:],
                                                     st["esum_q"][:])
                                nc.tensor.matmul(
                                    st["ps_r"][:], lhsT=ones_sb[:],
                                    rhs=st["esum_o"][:],
                                    start=(pr == 3), stop=False)
                            else:
                                nc.tensor.matmul(
                                    st["ps_r"][:], lhsT=ones_sb[:],
                                    rhs=st["esum_q"][:],
                                    start=(pr == 1), stop=False)

                    def diag_scores(dp):
                        # two diagonal chunks share one PSUM tile / one exp;
                        # the odd chunk's scores matmul is widened to the even
                        # offset so the exp never reads unwritten PSUM (the
                        # extra columns are above the diagonal and excluded
                        # from the ragged AV/rowsum reads below)
                        off_e = 2 * dp * P
                        ps2 = psS2.tile([P, 2, TQ], F32, tag="psscore",
                                        name="ps2d")
                        for s in range(2):
                            j = nfull + 2 * dp + s
                            nc.tensor.matmul(
                                ps2[:, s, off_e:TQ],
                                lhsT=kr[:, j * P:(j + 1) * P],
                                rhs=qr[:, qo * TQ + off_e:(qo + 1) * TQ],
                                start=True, stop=True)
                        et2 = etp.tile([P, 2, TQ], BF16, tag="et2",
                                       name="etd", bufs=5)
                        nc.scalar.activation(et2[:, :, off_e:TQ],
                                             ps2[:, :, off_e:TQ], AF.Exp,
                                             scale=SCALE)
                        for s in range(2):
                            off = (2 * dp + s) * P
                            nc.vector.tensor_mul(et2[:, s, off:off + P],
                                                 et2[:, s, off:off + P],
                                                 mask_sb[:])
                        return et2

                    def diag_av(dp, et2):
                        for s in range(2):
                            jr = 2 * dp + s
                            j = nfull + jr
                            off = jr * P
                            first = (jr == 0) and nfull == 0
                            nc.tensor.matmul(
                                ps_o[:, off:TQ], lhsT=v_b[h][:, j, :],
                                rhs=et2[:, s, off:TQ],
                                start=first, stop=(jr == 3))

                    def diag_esum_adds(etd0, etd1):
                        # fold all four diagonal E chunks into etd0 plane 0
                        # (in place, after its own AV has read the originals)
                        # so one full-width ones-matmul covers the diagonal
                        e0 = etd0[:, 0, :]
                        nc.vector.tensor_add(e0[:, P:TQ], e0[:, P:TQ],
                                             etd0[:, 1, P:TQ])
                        nc.vector.tensor_add(e0[:, 2 * P:TQ],
                                             e0[:, 2 * P:TQ],
                                             etd1[:, 0, 2 * P:TQ])
                        nc.vector.tensor_add(e0[:, 3 * P:TQ],
                                             e0[:, 3 * P:TQ],
                                             etd1[:, 1, 3 * P:TQ])

                    def diag_rowsum_mm(etd0):
                        nc.tensor.matmul(
                            st["ps_r"][:], lhsT=ones_sb[:],
                            rhs=etd0[:, 0, :],
                            start=(nfull == 0), stop=True)

                    if nq2 > 0:
                        # up to two pairs scored ahead (the psS2 ring depth);
                        # their exps complete under the interleaved proj tile
                        nahead = min(nq2, 2)
                        queue = [scores_pair(p, split_exp=(p == 0 and
                                                           proj_emit is None))
                                 for p in range(nahead)]
                        if proj_emit:
                            proj_emit()
                        # ps_r allocated after the proj tiles so the shared
                        # ring never waits across the current block
                        st["ps_r"] = psPj.tile([P, TQ], F32, tag="psproj",
                                               name="ps_r")
                        for pr in range(nahead, nq2):
                            queue.append(scores_pair(pr))
                            av_pair(pr - nahead, queue.pop(0))
                        if len(queue) == 2:
                            av_pair(nq2 - 2, queue.pop(0))
                        etd0 = diag_scores(0)
                        av_pair(nq2 - 1, queue.pop(0))
                        etd1 = diag_scores(1)
                        diag_av(0, etd0)
                        diag_esum_adds(etd0, etd1)
                        diag_av(1, etd1)
                        diag_rowsum_mm(etd0)
                    else:
                        etd0 = diag_scores(0)
                        if proj_emit:
                            proj_emit()
                        st["ps_r"] = psPj.tile([P, TQ], F32, tag="psproj",
                                               name="ps_r")
                        etd1 = diag_scores(1)
                        diag_av(0, etd0)
                        diag_esum_adds(etd0, etd1)
                        diag_av(1, etd1)
                        diag_rowsum_mm(etd0)
                    # rowsums arrive replicated on all partitions (ones lhsT)
                    recip = nrm.tile([P, TQ], F32, tag="recip", name="recip")
                    nc.vector.reciprocal_approx_fast(recip[:], st["ps_r"][:])
                    nc.vector.tensor_mul(outT[qo][:, h, :], ps_o[:], recip[:])

                def proj_tile(qo, tloc):
                    t = qo * NQ + tloc
                    ys = ystg.tile([P, T], F16, tag="ystage", name="ys")
                    for cn in range(NQ):
                        ps = psPj.tile([P, TQ], F32, tag="psproj", name="psy")
                        for h in range(NH):
                            nc.tensor.matmul(
                                ps[:],
                                lhsT=outT[qo][:, h, tloc * P:(tloc + 1) * P],
                                rhs=wp_b[:, h, cn * TQ:(cn + 1) * TQ],
                                start=(h == 0), stop=(h == NH - 1))
                        csl = slice(cn * TQ, (cn + 1) * TQ)
                        if cn < 3:
                            nc.vector.tensor_copy(ys[:, csl], ps[:])
                        else:
                            nc.scalar.activation(ys[:, csl], ps[:], AF.Copy)
                        if qo == 0 and tloc == NQ - 1:
                            # the final tile is the kernel tail: drain each
                            # quarter as soon as it is evacuated
                            eng = (nc.sync, nc.scalar)[cn % 2]
                            eng.dma_start(y[t * P:(t + 1) * P, csl],
                                          ys[:, csl])
                        elif cn % 2 == 1:
                            # half-row DMA: 2KB contiguous per partition
                            hsl = slice((cn - 1) * TQ, (cn + 1) * TQ)
                            eng = nc.sync if (t + cn) % 4 < 2 else nc.scalar
                            eng.dma_start(y[t * P:(t + 1) * P, hsl],
                                          ys[:, hsl])

                qos = [3, 2, 1, 0]
                for i, qo in enumerate(qos):
                    outT[qo] = ap_.tile([P, NH, TQ], BF16, tag="outT",
                                        name=f"outT{qo}", bufs=3)
                    for h in range(NH):
                        if i > 0:
                            pq, ph = qos[i - 1], h
                            attn_block(h, qo,
                                       lambda q=pq, t=ph: proj_tile(q, t))
                        else:
                            attn_block(h, qo)
                for tloc in range(NQ):
                    proj_tile(0, tloc)

    nc.compile()
    return nc


def _get_nc():
    global _CACHED_NC
    if _CACHED_NC is None:
        _CACHED_NC = build_nc()
    return _CACHED_NC


LAST_RESULTS = None


def kernel(x, cos, sin, W_attn, W_proj):
    global LAST_RESULTS
    x = np.asarray(x, np.float32)
    cos = np.asarray(cos, np.float32)
    sin = np.asarray(sin, np.float32)
    W_attn = np.asarray(W_attn, np.float32)
    W_proj = np.asarray(W_proj, np.float32)
    B = x.shape[0]

    cosT = np.ascontiguousarray(cos.T).astype(BF)          # [D, T]
    sinTf = np.ascontiguousarray(sin.T).copy()
    sinTf[: D // 2] *= -1.0                                # sign-folded rotate
    sinT = sinTf.astype(BF)

    xTs = [np.ascontiguousarray(x[b].T).astype(BF) for b in range(B)]
    in_maps = []
    for b in range(B):
        for g in range(4):
            csl = slice(g * 512, (g + 1) * 512)
            wqk2 = np.concatenate([W_attn[:, csl], W_attn[:, C:][:, csl]],
                                  axis=1).astype(BF)       # [C, 1024]
            # pack [8, 128, 16, 128]: wqkr[m, p, ko, j] = wqk2[128*ko+p, 128*m+j]
            wqkr = np.ascontiguousarray(
                wqk2.reshape(KO, P, 8, P).transpose(2, 1, 0, 3))
            wv2 = W_attn[:, 2 * C:][:, csl].astype(BF)     # [C, 512]
            wvr = np.ascontiguousarray(
                wv2.reshape(KO, P, NH * D).transpose(1, 0, 2))  # [128,16,512]
            wp2 = W_proj[g * 512:(g + 1) * 512, :].astype(BF)   # [512, C]
            wpr = np.ascontiguousarray(
                wp2.reshape(NH, P, C).transpose(1, 0, 2))       # [128,4,2048]
            in_maps.append({"xT": xTs[b], "wqk": wqkr, "wv": wvr, "wp": wpr,
                            "cosT": cosT, "sinT": sinT})

    nc = _get_nc()
    res = run_bass_kernel_spmd(nc, in_maps, core_ids=list(range(8)),
                               trace=TRACE)
    LAST_RESULTS = res

    out = np.zeros((B, T, C), np.float32)
    for b in range(B):
        acc = res.results[b * 4 + 0]["y"].astype(np.float32)
        for g in range(1, 4):
            acc = acc + res.results[b * 4 + g]["y"].astype(np.float32)
        out[b] = acc
    return out


# revision 48
# speedup vs baseline: 1.0006x; 1.0006x over previous
# Multi-head self-attention (B=2, T=2048, C=2048, H=16) on 8 trn2 NeuronCores.
# Sharding: core = (batch b, head-group g) with 4 heads per core.
# Inputs are pre-cast to bf16 and packed DMA-friendly on the host (the device
# would do the identical round-to-nearest cast before its bf16 matmuls).
# Per-core program (Tile framework, bf16 matmuls with fp32 PSUM accumulation):
#   warmup matmuls on a zero tile pre-warm the PE HAM clock gate while DMA
#     rings spin up
#   qk^T = W_qk^T @ x^T   (lhsT = W chunks, rhs = x^T)      -> [D, T] per head
#   v    = x @ W_v        (lhsT = x^T chunks, rhs = W_v)    -> [T, D] natural
#   RoPE on q^T/k^T via half-swap DMA + elementwise mul/add (in place)
#   merged attention+projection phase, qo-major [3,2,1,0]:
#     S^T tile = k_rope^T.T @ q_rope^T ; E^T = exp(scale*S^T) (causal)
#     out^T = v.T @ E^T ; rowsums: DVE-accumulate E tiles in fp32 SBUF,
#       one ones-matmul per (h,qo) replicates sums to all partitions
#     normalize off the PSUM path: evac unscaled, scale by 1/sums in SBUF
#     proj tiles of block qo interleave with attention of block qo-1 so the
#       PE stream stays dense and the y out-DMA is spread across the phase
#   y_partial = out_heads^T.T @ W_p rows  -> [T, C] fp16, host sums 4 partials.
import sys

import numpy as np
import ml_dtypes

for _p in ("/opt/trn_rl_repo",):
    if _p not in sys.path:
        sys.path.append(_p)

import concourse.bass as bass
import concourse.bass_isa as bass_isa
import concourse.mybir as mybir
import concourse.tile as tile
from concourse import bacc
from concourse.bass_utils import run_bass_kernel_spmd

P = 128
T = 2048
C = 2048
D = 128
NH = 4            # heads per core
KO = C // P       # 16 contraction chunks
TQ = 512          # q-tile width
NQ = T // TQ      # 4
NT = T // P       # 16 t-subtiles
SCALE = float(np.float32(1.0) / np.sqrt(np.float32(D)))

F32 = mybir.dt.float32
F16 = mybir.dt.float16
BF16 = mybir.dt.bfloat16
AF = mybir.ActivationFunctionType
BF = ml_dtypes.bfloat16

TRACE = False
_CACHED_NC = None


def _tri_mask_np():
    p = np.arange(P)[:, None]
    q = np.arange(P)[None, :]
    return (p <= q).astype(BF)


def build_nc():
    nc = bacc.Bacc("TRN2", target_bir_lowering=False, debug=False,
                   enable_asserts=False)

    # bf16 inputs, packed so every DMA moves >=4KB contiguous per partition
    xT_d = nc.dram_tensor("xT", [C, T], BF16, kind="ExternalInput")
    wqk_d = nc.dram_tensor("wqk", [8, P, KO, P], BF16, kind="ExternalInput")
    wv_d = nc.dram_tensor("wv", [P, KO, NH * D], BF16, kind="ExternalInput")
    wp_d = nc.dram_tensor("wp", [P, NH, C], BF16, kind="ExternalInput")
    cos_d = nc.dram_tensor("cosT", [D, T], BF16, kind="ExternalInput")
    sin_d = nc.dram_tensor("sinT", [D, T], BF16, kind="ExternalInput")
    y_d = nc.dram_tensor("y", [T, C], F16, kind="ExternalOutput")

    mask_d = nc.inline_tensor(_tri_mask_np(), name="trimask")

    xT = xT_d.ap().rearrange("(ko p) t -> p ko t", p=P)          # [128,16,2048]
    wqk = wqk_d.ap()
    y = y_d.ap()

    with tile.TileContext(nc) as tc:
        with (
            tc.tile_pool(name="glob", bufs=1) as glob,
            tc.tile_pool(name="rawp", bufs=1) as rawp,
        ):
            # HAM warmup: ~20 N=256 matmuls on a zeroed tile keep the PE busy
            # from ~t=2us so the clock gate is at 8/8 before real work arrives
            warm_sb = glob.tile([P, 256], BF16, tag="warm")
            nc.vector.memset(warm_sb[:], 0.0)
            ones_sb = glob.tile([P, P], BF16, tag="ones")
            nc.vector.memset(ones_sb[:], 1.0)
            v_b = [glob.tile([P, NT, P], BF16, tag=f"v_b{h}", name=f"v_b{h}")
                   for h in range(NH)]
            raw = [rawp.tile([P, T], BF16, tag=f"raw{m}", name=f"raw{m}")
                   for m in range(8)]
            mask_sb = glob.tile([P, P], BF16, tag="trimask")
            wp_b = glob.tile([P, NH, C], BF16, tag="wp_b")

            # =============== Phase B: qkv matmuls + RoPE ===============
            # warmps persists through phase B: its bank hosts the HAM-warmup
            # matmuls plus no-op filler matmuls that soak up PE idle while the
            # x chunks stream in
            with tc.tile_pool(name="warmps", bufs=1, space="PSUM") as wps, \
                 tc.tile_pool(name="loadB", bufs=1) as lB, \
                 tc.tile_pool(name="shufp", bufs=1) as shufp, \
                 tc.tile_pool(name="psB", bufs=7, space="PSUM") as psB:

                w_ps = wps.tile([P, 256], F32, tag="wps")
                for _ in range(17):
                    nc.tensor.matmul(w_ps[:], lhsT=warm_sb[:, 0:P],
                                     rhs=warm_sb[:], start=True, stop=True)

                def filler(dep=None, n=256):
                    # anchor on `dep` so the scheduler places the filler where
                    # that data lands instead of hoisting it to t=0
                    rhs = warm_sb[:, 0:n] if dep is None else dep
                    nc.tensor.matmul(w_ps[:, 0:rhs.shape[-1]],
                                     lhsT=warm_sb[:, 0:P],
                                     rhs=rhs, start=True, stop=True)

                xb = lB.tile([P, KO, T], BF16, tag="xb")
                wqk_b = [lB.tile([P, KO, P], BF16, tag=f"wqk_b{m}",
                                 name=f"wqk_b{m}") for m in range(8)]
                # x is the critical stream: only the first two m-groups'
                # weights jump the queue (groups 0-1 are all the PE can keep
                # in flight while x lands); everything else loads after x
                nc.sync.dma_start(wqk_b[0][:, 0:4, :], wqk[0, :, 0:4, :])
                nc.sync.dma_start(xb[:, 0, :], xT[:, 0, :])
                nc.sync.dma_start(wqk_b[0][:, 4:KO, :], wqk[0, :, 4:KO, :])
                nc.sync.dma_start(xb[:, 1, :], xT[:, 1, :])
                nc.sync.dma_start(wqk_b[1][:], wqk[1])
                for ko in range(2, KO):
                    nc.sync.dma_start(xb[:, ko, :], xT[:, ko, :])
                for m in range(2, 8):
                    nc.sync.dma_start(wqk_b[m][:], wqk[m])
                cos_b = lB.tile([P, T], BF16, tag="cos_b")
                sin_b = lB.tile([P, T], BF16, tag="sin_b")
                nc.sync.dma_start(cos_b[:], cos_d.ap())
                nc.sync.dma_start(sin_b[:], sin_d.ap())
                wv_b = lB.tile([P, KO, NH * D], BF16, tag="wv_b")
                nc.sync.dma_start(wv_b[:], wv_d.ap())
                nc.sync.dma_start(mask_sb[:], mask_d.ap())
                # wp is only needed by the first proj tile (~mid-kernel); queue
                # it after every qkv input so it can't delay the x/wqk stream
                nc.sync.dma_start(wp_b[:], wp_d.ap())

                # qk matmuls; k-order staggered by m so each group consumes
                # chunks roughly in DMA-arrival order
                for m in range(8):
                    pss = [psB.tile([P, TQ], F32, tag="psBig", name="psqk")
                           for _ in range(NQ)]
                    kos = [(m + i) % KO for i in range(KO)]
                    for i, ko in enumerate(kos):
                        for n in range(NQ):
                            nc.tensor.matmul(
                                pss[n][:], lhsT=wqk_b[m][:, ko, :],
                                rhs=xb[:, ko, n * TQ:(n + 1) * TQ],
                                start=(i == 0), stop=(i == KO - 1))
                        if m < 2 and i % 2 == 1:
                            filler(dep=xb[:, ko, 0:256])
                    for n in range(NQ):
                        nc.scalar.activation(
                            raw[m][:, n * TQ:(n + 1) * TQ], pss[n][:], AF.Copy)
                    # rope in place
                    r = raw[m]
                    sh = shufp.tile([P, T], BF16, tag="shuf", name="sh")
                    nc.sync.dma_start(sh[0:64, :], r[64:128, :])
                    nc.sync.dma_start(sh[64:128, :], r[0:64, :])
                    nc.vector.tensor_mul(sh[:], sh[:], sin_b[:])
                    nc.vector.tensor_mul(r[:], r[:], cos_b[:])
                    nc.vector.tensor_add(r[:], r[:], sh[:])

                # v matmuls (natural layout)
                for t in range(NT):
                    psv = psB.tile([P, TQ], F32, tag="psBig", name="psv")
                    for ko in range(KO):
                        nc.tensor.matmul(
                            psv[:], lhsT=xb[:, ko, t * P:(t + 1) * P],
                            rhs=wv_b[:, ko, :],
                            start=(ko == 0), stop=(ko == KO - 1))
                    for h in range(NH):
                        # split the last tiles' evacs across engines so the
                        # PSUM pool handoff to the attention phase is quick
                        if t >= NT - 2 and h < 2:
                            nc.vector.tensor_copy(
                                v_b[h][:, t, :], psv[:, h * P:(h + 1) * P])
                        else:
                            nc.scalar.activation(
                                v_b[h][:, t, :], psv[:, h * P:(h + 1) * P],
                                AF.Copy)
                # keep the PE busy across the pool transition into attention;
                # anchored on the last v tiles so these run at the handoff
                for h in range(NH):
                    filler(dep=v_b[h][:, NT - 1, :])
                    filler(dep=v_b[h][:, NT - 2, :])

            # =============== Merged attention + projection ===============
            with tc.tile_pool(name="attnp", bufs=1) as ap_, \
                 tc.tile_pool(name="etp", bufs=3) as etp, \
                 tc.tile_pool(name="nrm", bufs=2) as nrm, \
                 tc.tile_pool(name="psS2", bufs=2, space="PSUM") as psS2, \
                 tc.tile_pool(name="psO", bufs=2, space="PSUM") as psO, \
                 tc.tile_pool(name="psPj", bufs=2, space="PSUM") as psPj, \
                 tc.tile_pool(name="ystg", bufs=3) as ystg:

                # outT per qo block: [P(d), NH, TQ]; double-buffered so proj
                # of block qo overlaps attention of block qo-1
                outT = {}

                def attn_block(h, qo, proj_emit=None):
                    # Software-pipelined block: scores for chunk-pair p+1 are
                    # emitted before the AV matmuls of pair p, so each exp's
                    # ~1.2us latency hides under the next pair's scores; the
                    # interleaved proj tile is emitted right after the first
                    # scores pair and absorbs its exp latency at block entry.
                    qr = raw[h]
                    kr = raw[4 + h]
                    qsl = slice(qo * TQ, (qo + 1) * TQ)
                    nfull = 4 * qo
                    nq2 = nfull // 2
                    ps_o = psO.tile([P, TQ], F32, tag="psout", name="ps_o")
                    st = {"ps_r": None, "esum_q": None}

                    def scores_pair(pr, split_exp=False):
                        ps2 = psS2.tile([P, 2, TQ], F32, tag="psscore",
                                        name="ps2")
                        for s in range(2):
                            j = 2 * pr + s
                            nc.tensor.matmul(
                                ps2[:, s, :],
                                lhsT=kr[:, j * P:(j + 1) * P],
                                rhs=qr[:, qsl], start=True, stop=True)
                        et2 = etp.tile([P, 2, TQ], BF16, tag="et2",
                                       name="et2", bufs=5)
                        if split_exp:
                            # per-plane exps: the first AV only needs plane 0,
                            # so it can start ~half an exp earlier (used at
                            # block entry when no proj tile covers the wait)
                            for s in range(2):
                                nc.scalar.activation(et2[:, s, :],
                                                     ps2[:, s, :], AF.Exp,
                                                     scale=SCALE)
                        else:
                            nc.scalar.activation(et2[:], ps2[:], AF.Exp,
                                                 scale=SCALE)
                        return et2

                    def av_pair(pr, et2):
                        for s in range(2):
                            j = 2 * pr + s
                            nc.tensor.matmul(
                                ps_o[:], lhsT=v_b[h][:, j, :],
                                rhs=et2[:, s, :],
                                start=(pr == 0 and s == 0), stop=False)
                        esum = etp.tile([P, TQ], BF16, tag="esum",
                                        name="esum", bufs=6)
                        nc.vector.tensor_add(esum[:], et2[:, 0, :],
                                             et2[:, 1, :])
                        if pr % 2 == 0:
                            st["esum_q"] = esum
                        else:
                            # quad complete; batch two quads into one oct
                            # rowsum matmul when a second quad is coming
                            nc.vector.tensor_add(st["esum_q"][:],
                                                 st["esum_q"][:], esum[:])
                            if pr % 4 == 1 and pr + 2 < nq2:
                                st["esum_o"] = st["esum_q"]
                            elif pr % 4 == 3:
                                nc.vector.tensor_add(st["esum_o"][:],
                                                     st["esum_o"][:],
                                                     st["esum_q"][:])
                                nc.tensor.matmul(
                                    st["ps_r"][:], lhsT=ones_sb[:],
                                    rhs=st["esum_o"][:],
                                    start=(pr == 3), stop=False)
                            else:
                                nc.tensor.matmul(
                                    st["ps_r"][:], lhsT=ones_sb[:],
                                    rhs=st["esum_q"][:],
                                    start=(pr == 1), stop=False)

                    def diag_scores(dp):
                        # two diagonal chunks share one PSUM tile / one exp;
                        # each chunk's scores matmul covers only its causal
                        # range, so the batched exp reads 128 stale PSUM
                        # columns on the odd plane — those land in an SBUF
                        # region excluded from every downstream ragged read
                        off_e = 2 * dp * P
                        ps2 = psS2.tile([P, 2, TQ], F32, tag="psscore",
                                        name="ps2d")
                        for s in range(2):
                            j = nfull + 2 * dp + s
                            off = (2 * dp + s) * P
                            nc.tensor.matmul(
                                ps2[:, s, off:TQ],
                                lhsT=kr[:, j * P:(j + 1) * P],
                                rhs=qr[:, qo * TQ + off:(qo + 1) * TQ],
                                start=True, stop=True)
                        et2 = etp.tile([P, 2, TQ], BF16, tag="et2",
                                       name="etd", bufs=5)
                        nc.scalar.activation(et2[:, :, off_e:TQ],
                                             ps2[:, :, off_e:TQ], AF.Exp,
                                             scale=SCALE)
                        for s in range(2):
                            off = (2 * dp + s) * P
                            nc.vector.tensor_mul(et2[:, s, off:off + P],
                                                 et2[:, s, off:off + P],
                                                 mask_sb[:])
                        return et2

                    def diag_av(dp, et2):
                        for s in range(2):
                            jr = 2 * dp + s
                            j = nfull + jr
                            off = jr * P
                            first = (jr == 0) and nfull == 0
                            nc.tensor.matmul(
                                ps_o[:, off:TQ], lhsT=v_b[h][:, j, :],
                                rhs=et2[:, s, off:TQ],
                                start=first, stop=(jr == 3))

                    def diag_esum_adds(etd0, etd1):
                        # fold all four diagonal E chunks into etd0 plane 0
                        # (in place, after its own AV has read the originals)
                        # so one full-width ones-matmul covers the diagonal
                        e0 = etd0[:, 0, :]
                        nc.vector.tensor_add(e0[:, P:TQ], e0[:, P:TQ],
                                             etd0[:, 1, P:TQ])
                        nc.vector.tensor_add(e0[:, 2 * P:TQ],
                                             e0[:, 2 * P:TQ],
                                             etd1[:, 0, 2 * P:TQ])
                        nc.vector.tensor_add(e0[:, 3 * P:TQ],
                                             e0[:, 3 * P:TQ],
                                             etd1[:, 1, 3 * P:TQ])

                    def diag_rowsum_mm(etd0):
                        nc.tensor.matmul(
                            st["ps_r"][:], lhsT=ones_sb[:],
                            rhs=etd0[:, 0, :],
                            start=(nfull == 0), stop=True)

                    if nq2 > 0:
                        # up to two pairs scored ahead (the psS2 ring depth);
                        # their exps complete under the interleaved proj tile
                        nahead = min(nq2, 2)
                        queue = [scores_pair(p, split_exp=(p == 0 and
                                                           proj_emit is None))
                                 for p in range(nahead)]
                        if proj_emit:
                            proj_emit()
                        # ps_r allocated after the proj tiles so the shared
                        # ring never waits across the current block
                        st["ps_r"] = psPj.tile([P, TQ], F32, tag="psproj",
                                               name="ps_r")
                        for pr in range(nahead, nq2):
                            queue.append(scores_pair(pr))
                            av_pair(pr - nahead, queue.pop(0))
                        if len(queue) == 2:
                            av_pair(nq2 - 2, queue.pop(0))
                        etd0 = diag_scores(0)
                        av_pair(nq2 - 1, queue.pop(0))
                        etd1 = diag_scores(1)
                        diag_av(0, etd0)
                        diag_esum_adds(etd0, etd1)
                        diag_av(1, etd1)
                        diag_rowsum_mm(etd0)
                    else:
                        etd0 = diag_scores(0)
                        if proj_emit:
                            proj_emit()
                        st["ps_r"] = psPj.tile([P, TQ], F32, tag="psproj",
                                               name="ps_r")
                        etd1 = diag_scores(1)
                        diag_av(0, etd0)
                        diag_esum_adds(etd0, etd1)
                        diag_av(1, etd1)
                        diag_rowsum_mm(etd0)
                    # rowsums arrive replicated on all partitions (ones lhsT)
                    recip = nrm.tile([P, TQ], F32, tag="recip", name="recip")
                    nc.vector.reciprocal_approx_fast(recip[:], st["ps_r"][:])
                    nc.vector.tensor_mul(outT[qo][:, h, :], ps_o[:], recip[:])

                def proj_tile(qo, tloc):
                    t = qo * NQ + tloc
                    ys = ystg.tile([P, T], F16, tag="ystage", name="ys")
                    for cn in range(NQ):
                        ps = psPj.tile([P, TQ], F32, tag="psproj", name="psy")
                        for h in range(NH):
                            nc.tensor.matmul(
                                ps[:],
                                lhsT=outT[qo][:, h, tloc * P:(tloc + 1) * P],
                                rhs=wp_b[:, h, cn * TQ:(cn + 1) * TQ],
                                start=(h == 0), stop=(h == NH - 1))
                        csl = slice(cn * TQ, (cn + 1) * TQ)
                        if cn < 3:
                            nc.vector.tensor_copy(ys[:, csl], ps[:])
                        else:
                            nc.scalar.activation(ys[:, csl], ps[:], AF.Copy)
                        if qo == 0 and tloc == NQ - 1:
                            # the final tile is the kernel tail: drain each
                            # quarter as soon as it is evacuated
                            eng = (nc.sync, nc.scalar)[cn % 2]
                            eng.dma_start(y[t * P:(t + 1) * P, csl],
                                          ys[:, csl])
                        elif cn % 2 == 1:
                            # half-row DMA: 2KB contiguous per partition
                            hsl = slice((cn - 1) * TQ, (cn + 1) * TQ)
                            eng = nc.sync if (t + cn) % 4 < 2 else nc.scalar
                            eng.dma_start(y[t * P:(t + 1) * P, hsl],
                                          ys[:, hsl])

                qos = [3, 2, 1, 0]
                for i, qo in enumerate(qos):
                    outT[qo] = ap_.tile([P, NH, TQ], BF16, tag="outT",
                                        name=f"outT{qo}", bufs=2)
                    for h in range(NH):
                        if i > 0:
                            pq, ph = qos[i - 1], h
                            attn_block(h, qo,
                                       lambda q=pq, t=ph: proj_tile(q, t))
                        else:
                            attn_block(h, qo)
                for tloc in range(NQ):
                    proj_tile(0, tloc)

    nc.compile()
    return nc


def _get_nc():
    global _CACHED_NC
    if _CACHED_NC is None:
        _CACHED_NC = build_nc()
    return _CACHED_NC


LAST_RESULTS = None


def kernel(x, cos, sin, W_attn, W_proj):
    global LAST_RESULTS
    x = np.asarray(x, np.float32)
    cos = np.asarray(cos, np.float32)
    sin = np.asarray(sin, np.float32)
    W_attn = np.asarray(W_attn, np.float32)
    W_proj = np.asarray(W_proj, np.float32)
    B = x.shape[0]

    cosT = np.ascontiguousarray(cos.T).astype(BF)          # [D, T]
    sinTf = np.ascontiguousarray(sin.T).copy()
    sinTf[: D // 2] *= -1.0                                # sign-folded rotate
    sinT = sinTf.astype(BF)

    xTs = [np.ascontiguousarray(x[b].T).astype(BF) for b in range(B)]
    in_maps = []
    for b in range(B):
        for g in range(4):
            csl = slice(g * 512, (g + 1) * 512)
            wqk2 = np.concatenate([W_attn[:, csl], W_attn[:, C:][:, csl]],
                                  axis=1).astype(BF)       # [C, 1024]
            # pack [8, 128, 16, 128]: wqkr[m, p, ko, j] = wqk2[128*ko+p, 128*m+j]
            wqkr = np.ascontiguousarray(
                wqk2.reshape(KO, P, 8, P).transpose(2, 1, 0, 3))
            wv2 = W_attn[:, 2 * C:][:, csl].astype(BF)     # [C, 512]
            wvr = np.ascontiguousarray(
                wv2.reshape(KO, P, NH * D).transpose(1, 0, 2))  # [128,16,512]
            wp2 = W_proj[g * 512:(g + 1) * 512, :].astype(BF)   # [512, C]
            wpr = np.ascontiguousarray(
                wp2.reshape(NH, P, C).transpose(1, 0, 2))       # [128,4,2048]
            in_maps.append({"xT": xTs[b], "wqk": wqkr, "wv": wvr, "wp": wpr,
                            "cosT": cosT, "sinT": sinT})

    nc = _get_nc()
    res = run_bass_kernel_spmd(nc, in_maps, core_ids=list(range(8)),
                               trace=TRACE)
    LAST_RESULTS = res

    out = np.zeros((B, T, C), np.float32)
    for b in range(B):
        acc = res.results[b * 4 + 0]["y"].astype(np.float32)
        for g in range(1, 4):
            acc = acc + res.results[b * 4 + g]["y"].astype(np.float32)
        out[b] = acc
    return out


# revision 49
# speedup vs baseline: 1.0026x; 1.0021x over previous
# Multi-head self-attention (B=2, T=2048, C=2048, H=16) on 8 trn2 NeuronCores.
# Sharding: core = (batch b, head-group g) with 4 heads per core.
# Inputs are pre-cast to bf16 and packed DMA-friendly on the host (the device
# would do the identical round-to-nearest cast before its bf16 matmuls).
# Per-core program (Tile framework, bf16 matmuls with fp32 PSUM accumulation):
#   warmup matmuls on a zero tile pre-warm the PE HAM clock gate while DMA
#     rings spin up
#   qk^T = W_qk^T @ x^T   (lhsT = W chunks, rhs = x^T)      -> [D, T] per head
#   v    = x @ W_v        (lhsT = x^T chunks, rhs = W_v)    -> [T, D] natural
#   RoPE on q^T/k^T via half-swap DMA + elementwise mul/add (in place)
#   merged attention+projection phase, qo-major [3,2,1,0]:
#     S^T tile = k_rope^T.T @ q_rope^T ; E^T = exp(scale*S^T) (causal)
#     out^T = v.T @ E^T ; rowsums: DVE-accumulate E tiles in fp32 SBUF,
#       one ones-matmul per (h,qo) replicates sums to all partitions
#     normalize off the PSUM path: evac unscaled, scale by 1/sums in SBUF
#     proj tiles of block qo interleave with attention of block qo-1 so the
#       PE stream stays dense and the y out-DMA is spread across the phase
#   y_partial = out_heads^T.T @ W_p rows  -> [T, C] fp16, host sums 4 partials.
import sys

import numpy as np
import ml_dtypes

for _p in ("/opt/trn_rl_repo",):
    if _p not in sys.path:
        sys.path.append(_p)

import concourse.bass as bass
import concourse.bass_isa as bass_isa
import concourse.mybir as mybir
import concourse.tile as tile
from concourse import bacc
from concourse.bass_utils import run_bass_kernel_spmd

P = 128
T = 2048
C = 2048
D = 128
NH = 4            # heads per core
KO = C // P       # 16 contraction chunks
TQ = 512          # q-tile width
NQ = T // TQ      # 4
NT = T // P       # 16 t-subtiles
SCALE = float(np.float32(1.0) / np.sqrt(np.float32(D)))

F32 = mybir.dt.float32
F16 = mybir.dt.float16
BF16 = mybir.dt.bfloat16
AF = mybir.ActivationFunctionType
BF = ml_dtypes.bfloat16

TRACE = False
_CACHED_NC = None


def _tri_mask_np():
    p = np.arange(P)[:, None]
    q = np.arange(P)[None, :]
    return (p <= q).astype(BF)


def build_nc():
    nc = bacc.Bacc("TRN2", target_bir_lowering=False, debug=False,
                   enable_asserts=False)

    # bf16 inputs, packed so every DMA moves >=4KB contiguous per partition
    xT_d = nc.dram_tensor("xT", [C, T], BF16, kind="ExternalInput")
    wqk_d = nc.dram_tensor("wqk", [8, P, KO, P], BF16, kind="ExternalInput")
    wv_d = nc.dram_tensor("wv", [P, KO, NH * D], BF16, kind="ExternalInput")
    wp_d = nc.dram_tensor("wp", [P, NH, C], BF16, kind="ExternalInput")
    cos_d = nc.dram_tensor("cosT", [D, T], BF16, kind="ExternalInput")
    sin_d = nc.dram_tensor("sinT", [D, T], BF16, kind="ExternalInput")
    y_d = nc.dram_tensor("y", [T, C], F16, kind="ExternalOutput")

    mask_d = nc.inline_tensor(_tri_mask_np(), name="trimask")

    xT = xT_d.ap().rearrange("(ko p) t -> p ko t", p=P)          # [128,16,2048]
    wqk = wqk_d.ap()
    y = y_d.ap()

    with tile.TileContext(nc) as tc:
        with (
            tc.tile_pool(name="glob", bufs=1) as glob,
            tc.tile_pool(name="rawp", bufs=1) as rawp,
        ):
            # HAM warmup: ~20 N=256 matmuls on a zeroed tile keep the PE busy
            # from ~t=2us so the clock gate is at 8/8 before real work arrives
            warm_sb = glob.tile([P, 256], BF16, tag="warm")
            nc.vector.memset(warm_sb[:], 0.0)
            ones_sb = glob.tile([P, P], BF16, tag="ones")
            nc.vector.memset(ones_sb[:], 1.0)
            v_b = [glob.tile([P, NT, P], BF16, tag=f"v_b{h}", name=f"v_b{h}")
                   for h in range(NH)]
            raw = [rawp.tile([P, T], BF16, tag=f"raw{m}", name=f"raw{m}")
                   for m in range(8)]
            mask_sb = glob.tile([P, P], BF16, tag="trimask")
            wp_b = glob.tile([P, NH, C], BF16, tag="wp_b")

            # =============== Phase B: qkv matmuls + RoPE ===============
            # warmps persists through phase B: its bank hosts the HAM-warmup
            # matmuls plus no-op filler matmuls that soak up PE idle while the
            # x chunks stream in
            with tc.tile_pool(name="warmps", bufs=1, space="PSUM") as wps, \
                 tc.tile_pool(name="loadB", bufs=1) as lB, \
                 tc.tile_pool(name="shufp", bufs=1) as shufp, \
                 tc.tile_pool(name="psB", bufs=7, space="PSUM") as psB:

                w_ps = wps.tile([P, 256], F32, tag="wps")
                for _ in range(17):
                    nc.tensor.matmul(w_ps[:], lhsT=warm_sb[:, 0:P],
                                     rhs=warm_sb[:], start=True, stop=True)

                def filler(dep=None, n=256):
                    # anchor on `dep` so the scheduler places the filler where
                    # that data lands instead of hoisting it to t=0
                    rhs = warm_sb[:, 0:n] if dep is None else dep
                    nc.tensor.matmul(w_ps[:, 0:rhs.shape[-1]],
                                     lhsT=warm_sb[:, 0:P],
                                     rhs=rhs, start=True, stop=True)

                xb = lB.tile([P, KO, T], BF16, tag="xb")
                wqk_b = [lB.tile([P, KO, P], BF16, tag=f"wqk_b{m}",
                                 name=f"wqk_b{m}") for m in range(8)]
                # x is the critical stream: only the first two m-groups'
                # weights jump the queue (groups 0-1 are all the PE can keep
                # in flight while x lands); everything else loads after x
                nc.sync.dma_start(wqk_b[0][:, 0:4, :], wqk[0, :, 0:4, :])
                nc.sync.dma_start(xb[:, 0, :], xT[:, 0, :])
                nc.sync.dma_start(wqk_b[0][:, 4:KO, :], wqk[0, :, 4:KO, :])
                nc.sync.dma_start(xb[:, 1, :], xT[:, 1, :])
                nc.sync.dma_start(wqk_b[1][:], wqk[1])
                for ko in range(2, KO):
                    nc.sync.dma_start(xb[:, ko, :], xT[:, ko, :])
                for m in range(2, 8):
                    nc.sync.dma_start(wqk_b[m][:], wqk[m])
                cos_b = lB.tile([P, T], BF16, tag="cos_b")
                sin_b = lB.tile([P, T], BF16, tag="sin_b")
                nc.sync.dma_start(cos_b[:], cos_d.ap())
                nc.sync.dma_start(sin_b[:], sin_d.ap())
                wv_b = lB.tile([P, KO, NH * D], BF16, tag="wv_b")
                nc.sync.dma_start(wv_b[:], wv_d.ap())
                nc.sync.dma_start(mask_sb[:], mask_d.ap())
                # wp is only needed by the first proj tile (~mid-kernel); queue
                # it after every qkv input so it can't delay the x/wqk stream
                nc.sync.dma_start(wp_b[:], wp_d.ap())

                # qk matmuls; k-order staggered by m so each group consumes
                # chunks roughly in DMA-arrival order
                for m in range(8):
                    pss = [psB.tile([P, TQ], F32, tag="psBig", name="psqk")
                           for _ in range(NQ)]
                    kos = [(m + i) % KO for i in range(KO)]
                    for i, ko in enumerate(kos):
                        for n in range(NQ):
                            nc.tensor.matmul(
                                pss[n][:], lhsT=wqk_b[m][:, ko, :],
                                rhs=xb[:, ko, n * TQ:(n + 1) * TQ],
                                start=(i == 0), stop=(i == KO - 1))
                        if m < 2 and i % 2 == 1:
                            filler(dep=xb[:, ko, 0:256])
                    for n in range(NQ):
                        nc.scalar.activation(
                            raw[m][:, n * TQ:(n + 1) * TQ], pss[n][:], AF.Copy)
                    # rope in place
                    r = raw[m]
                    sh = shufp.tile([P, T], BF16, tag="shuf", name="sh")
                    nc.sync.dma_start(sh[0:64, :], r[64:128, :])
                    nc.sync.dma_start(sh[64:128, :], r[0:64, :])
                    nc.vector.tensor_mul(sh[:], sh[:], sin_b[:])
                    nc.vector.tensor_mul(r[:], r[:], cos_b[:])
                    nc.vector.tensor_add(r[:], r[:], sh[:])

                # v matmuls (natural layout)
                for t in range(NT):
                    psv = psB.tile([P, TQ], F32, tag="psBig", name="psv")
                    for ko in range(KO):
                        nc.tensor.matmul(
                            psv[:], lhsT=xb[:, ko, t * P:(t + 1) * P],
                            rhs=wv_b[:, ko, :],
                            start=(ko == 0), stop=(ko == KO - 1))
                    for h in range(NH):
                        # split the last tiles' evacs across engines so the
                        # PSUM pool handoff to the attention phase is quick
                        if t >= NT - 2 and h < 2:
                            nc.vector.tensor_copy(
                                v_b[h][:, t, :], psv[:, h * P:(h + 1) * P])
                        else:
                            nc.scalar.activation(
                                v_b[h][:, t, :], psv[:, h * P:(h + 1) * P],
                                AF.Copy)
                # keep the PE busy across the pool transition into attention;
                # anchored on the last v tiles so these run at the handoff
                for h in range(NH):
                    filler(dep=v_b[h][:, NT - 1, :])
                    filler(dep=v_b[h][:, NT - 2, :])

            # =============== Merged attention + projection ===============
            with tc.tile_pool(name="attnp", bufs=1) as ap_, \
                 tc.tile_pool(name="etp", bufs=3) as etp, \
                 tc.tile_pool(name="nrm", bufs=2) as nrm, \
                 tc.tile_pool(name="psS2", bufs=2, space="PSUM") as psS2, \
                 tc.tile_pool(name="psO", bufs=2, space="PSUM") as psO, \
                 tc.tile_pool(name="psPj", bufs=2, space="PSUM") as psPj, \
                 tc.tile_pool(name="ystg", bufs=3) as ystg:

                # outT per qo block: [P(d), NH, TQ]; double-buffered so proj
                # of block qo overlaps attention of block qo-1
                outT = {}

                def attn_block(h, qo, proj_emit=None):
                    # Software-pipelined block: scores for chunk-pair p+1 are
                    # emitted before the AV matmuls of pair p, so each exp's
                    # ~1.2us latency hides under the next pair's scores; the
                    # interleaved proj tile is emitted right after the first
                    # scores pair and absorbs its exp latency at block entry.
                    qr = raw[h]
                    kr = raw[4 + h]
                    qsl = slice(qo * TQ, (qo + 1) * TQ)
                    nfull = 4 * qo
                    nq2 = nfull // 2
                    ps_o = psO.tile([P, TQ], F32, tag="psout", name="ps_o")
                    st = {"ps_r": None, "esum_q": None}

                    def scores_pair(pr, split_exp=False):
                        ps2 = psS2.tile([P, 2, TQ], F32, tag="psscore",
                                        name="ps2")
                        for s in range(2):
                            j = 2 * pr + s
                            nc.tensor.matmul(
                                ps2[:, s, :],
                                lhsT=kr[:, j * P:(j + 1) * P],
                                rhs=qr[:, qsl], start=True, stop=True)
                        et2 = etp.tile([P, 2, TQ], BF16, tag="et2",
                                       name="et2", bufs=5)
                        if split_exp:
                            # per-plane exps: the first AV only needs plane 0,
                            # so it can start ~half an exp earlier (used at
                            # block entry when no proj tile covers the wait)
                            for s in range(2):
                                nc.scalar.activation(et2[:, s, :],
                                                     ps2[:, s, :], AF.Exp,
                                                     scale=SCALE)
                        else:
                            nc.scalar.activation(et2[:], ps2[:], AF.Exp,
                                                 scale=SCALE)
                        return et2

                    def av_pair(pr, et2):
                        for s in range(2):
                            j = 2 * pr + s
                            nc.tensor.matmul(
                                ps_o[:], lhsT=v_b[h][:, j, :],
                                rhs=et2[:, s, :],
                                start=(pr == 0 and s == 0), stop=False)
                        esum = etp.tile([P, TQ], BF16, tag="esum",
                                        name="esum", bufs=6)
                        nc.vector.tensor_add(esum[:], et2[:, 0, :],
                                             et2[:, 1, :])
                        if pr % 2 == 0:
                            st["esum_q"] = esum
                        else:
                            # quad complete; batch two quads into one oct
                            # rowsum matmul when a second quad is coming
                            nc.vector.tensor_add(st["esum_q"][:],
                                                 st["esum_q"][:], esum[:])
                            if pr % 4 == 1 and pr + 2 < nq2:
                                st["esum_o"] = st["esum_q"]
                            elif pr % 4 == 3:
                                nc.vector.tensor_add(st["esum_o"][:],
                                                     st["esum_o"][:],
                                                     st["esum_q"][:])
                                nc.tensor.matmul(
                                    st["ps_r"][:], lhsT=ones_sb[:],
                                    rhs=st["esum_o"][:],
                                    start=(pr == 3), stop=False)
                            else:
                                nc.tensor.matmul(
                                    st["ps_r"][:], lhsT=ones_sb[:],
                                    rhs=st["esum_q"][:],
                                    start=(pr == 1), stop=False)

                    def diag_scores(dp):
                        # two diagonal chunks share one PSUM tile / one exp;
                        # the odd chunk's scores matmul is widened to the even
                        # offset so the exp never reads unwritten PSUM (the
                        # extra columns are above the diagonal and excluded
                        # from the ragged AV/rowsum reads below)
                        off_e = 2 * dp * P
                        ps2 = psS2.tile([P, 2, TQ], F32, tag="psscore",
                                        name="ps2d")
                        for s in range(2):
                            j = nfull + 2 * dp + s
                            nc.tensor.matmul(
                                ps2[:, s, off_e:TQ],
                                lhsT=kr[:, j * P:(j + 1) * P],
                                rhs=qr[:, qo * TQ + off_e:(qo + 1) * TQ],
                                start=True, stop=True)
                        et2 = etp.tile([P, 2, TQ], BF16, tag="et2",
                                       name="etd", bufs=5)
                        nc.scalar.activation(et2[:, :, off_e:TQ],
                                             ps2[:, :, off_e:TQ], AF.Exp,
                                             scale=SCALE)
                        for s in range(2):
                            off = (2 * dp + s) * P
                            nc.vector.tensor_mul(et2[:, s, off:off + P],
                                                 et2[:, s, off:off + P],
                                                 mask_sb[:])
                        return et2

                    def diag_av(dp, et2):
                        for s in range(2):
                            jr = 2 * dp + s
                            j = nfull + jr
                            off = jr * P
                            first = (jr == 0) and nfull == 0
                            nc.tensor.matmul(
                                ps_o[:, off:TQ], lhsT=v_b[h][:, j, :],
                                rhs=et2[:, s, off:TQ],
                                start=first, stop=(jr == 3))

                    def diag_esum_adds(etd0, etd1):
                        # fold all four diagonal E chunks into etd0 plane 0
                        # (in place, after its own AV has read the originals)
                        # so one full-width ones-matmul covers the diagonal
                        e0 = etd0[:, 0, :]
                        nc.vector.tensor_add(e0[:, P:TQ], e0[:, P:TQ],
                                             etd0[:, 1, P:TQ])
                        nc.vector.tensor_add(e0[:, 2 * P:TQ],
                                             e0[:, 2 * P:TQ],
                                             etd1[:, 0, 2 * P:TQ])
                        nc.vector.tensor_add(e0[:, 3 * P:TQ],
                                             e0[:, 3 * P:TQ],
                                             etd1[:, 1, 3 * P:TQ])

                    def diag_rowsum_mm(etd0):
                        nc.tensor.matmul(
                            st["ps_r"][:], lhsT=ones_sb[:],
                            rhs=etd0[:, 0, :],
                            start=(nfull == 0), stop=True)

                    if nq2 > 0:
                        # up to two pairs scored ahead (the psS2 ring depth);
                        # their exps complete under the interleaved proj tile
                        nahead = min(nq2, 2)
                        queue = [scores_pair(p, split_exp=(p == 0 and
                                                           proj_emit is None))
                                 for p in range(nahead)]
                        if proj_emit:
                            proj_emit()
                        # ps_r allocated after the proj tiles so the shared
                        # ring never waits across the current block
                        st["ps_r"] = psPj.tile([P, TQ], F32, tag="psproj",
                                               name="ps_r")
                        for pr in range(nahead, nq2):
                            queue.append(scores_pair(pr))
                            av_pair(pr - nahead, queue.pop(0))
                        if len(queue) == 2:
                            av_pair(nq2 - 2, queue.pop(0))
                        etd0 = diag_scores(0)
                        av_pair(nq2 - 1, queue.pop(0))
                        etd1 = diag_scores(1)
                        diag_av(0, etd0)
                        diag_esum_adds(etd0, etd1)
                        diag_av(1, etd1)
                        diag_rowsum_mm(etd0)
                    else:
                        etd0 = diag_scores(0)
                        if proj_emit:
                            proj_emit()
                        st["ps_r"] = psPj.tile([P, TQ], F32, tag="psproj",
                                               name="ps_r")
                        etd1 = diag_scores(1)
                        diag_av(0, etd0)
                        diag_esum_adds(etd0, etd1)
                        diag_av(1, etd1)
                        diag_rowsum_mm(etd0)
                    # rowsums arrive replicated on all partitions (ones lhsT)
                    recip = nrm.tile([P, TQ], F32, tag="recip", name="recip")
                    nc.vector.reciprocal_approx_fast(recip[:], st["ps_r"][:])
                    nc.vector.tensor_mul(outT[qo][:, h, :], ps_o[:], recip[:])

                def proj_tile(qo, tloc):
                    t = qo * NQ + tloc
                    ys = ystg.tile([P, T], F16, tag="ystage", name="ys")
                    for cn in range(NQ):
                        ps = psPj.tile([P, TQ], F32, tag="psproj", name="psy")
                        for h in range(NH):
                            nc.tensor.matmul(
                                ps[:],
                                lhsT=outT[qo][:, h, tloc * P:(tloc + 1) * P],
                                rhs=wp_b[:, h, cn * TQ:(cn + 1) * TQ],
                                start=(h == 0), stop=(h == NH - 1))
                        csl = slice(cn * TQ, (cn + 1) * TQ)
                        if cn < 3:
                            nc.vector.tensor_copy(ys[:, csl], ps[:])
                        else:
                            nc.scalar.activation(ys[:, csl], ps[:], AF.Copy)
                        if qo == 0 and tloc == NQ - 1:
                            # the final tile is the kernel tail: drain each
                            # quarter as soon as it is evacuated
                            eng = (nc.sync, nc.scalar)[cn % 2]
                            eng.dma_start(y[t * P:(t + 1) * P, csl],
                                          ys[:, csl])
                        elif cn % 2 == 1:
                            # half-row DMA: 2KB contiguous per partition
                            hsl = slice((cn - 1) * TQ, (cn + 1) * TQ)
                            eng = nc.sync if (t + cn) % 4 < 2 else nc.scalar
                            eng.dma_start(y[t * P:(t + 1) * P, hsl],
                                          ys[:, hsl])

                qos = [3, 2, 1, 0]
                for i, qo in enumerate(qos):
                    outT[qo] = ap_.tile([P, NH, TQ], BF16, tag="outT",
                                        name=f"outT{qo}", bufs=2)
                    for h in range(NH):
                        if i > 0:
                            pq, ph = qos[i - 1], h
                            attn_block(h, qo,
                                       lambda q=pq, t=ph: proj_tile(q, t))
                        else:
                            attn_block(h, qo)
                for tloc in range(NQ):
                    proj_tile(0, tloc)

    nc.compile()
    return nc


def _get_nc():
    global _CACHED_NC
    if _CACHED_NC is None:
        _CACHED_NC = build_nc()
    return _CACHED_NC


LAST_RESULTS = None


def kernel(x, cos, sin, W_attn, W_proj):
    global LAST_RESULTS
    x = np.asarray(x, np.float32)
    cos = np.asarray(cos, np.float32)
    sin = np.asarray(sin, np.float32)
    W_attn = np.asarray(W_attn, np.float32)
    W_proj = np.asarray(W_proj, np.float32)
    B = x.shape[0]

    cosT = np.ascontiguousarray(cos.T).astype(BF)          # [D, T]
    sinTf = np.ascontiguousarray(sin.T).copy()
    sinTf[: D // 2] *= -1.0                                # sign-folded rotate
    sinT = sinTf.astype(BF)

    xTs = [np.ascontiguousarray(x[b].T).astype(BF) for b in range(B)]
    in_maps = []
    for b in range(B):
        for g in range(4):
            csl = slice(g * 512, (g + 1) * 512)
            wqk2 = np.concatenate([W_attn[:, csl], W_attn[:, C:][:, csl]],
                                  axis=1).astype(BF)       # [C, 1024]
            # pack [8, 128, 16, 128]: wqkr[m, p, ko, j] = wqk2[128*ko+p, 128*m+j]
            wqkr = np.ascontiguousarray(
                wqk2.reshape(KO, P, 8, P).transpose(2, 1, 0, 3))
            wv2 = W_attn[:, 2 * C:][:, csl].astype(BF)     # [C, 512]
            wvr = np.ascontiguousarray(
                wv2.reshape(KO, P, NH * D).transpose(1, 0, 2))  # [128,16,512]
            wp2 = W_proj[g * 512:(g + 1) * 512, :].astype(BF)   # [512, C]
            wpr = np.ascontiguousarray(
                wp2.reshape(NH, P, C).transpose(1, 0, 2))       # [128,4,2048]
            in_maps.append({"xT": xTs[b], "wqk": wqkr, "wv": wvr, "wp": wpr,
                            "cosT": cosT, "sinT": sinT})

    nc = _get_nc()
    res = run_bass_kernel_spmd(nc, in_maps, core_ids=list(range(8)),
                               trace=TRACE)
    LAST_RESULTS = res

    out = np.zeros((B, T, C), np.float32)
    for b in range(B):
        acc = res.results[b * 4 + 0]["y"].astype(np.float32)
        for g in range(1, 4):
            acc = acc + res.results[b * 4 + g]["y"].astype(np.float32)
        out[b] = acc
    return out
